# revision 18
# baseline (speedup 1.0000x reference)
"""BlockDiagonalLowRankLinear Trainium2 kernel.

y = BlockDiag(blocks) @ x + U @ (V.T @ x), scaled by alpha, plus bias.

Shapes (full problem):
  x      [4, 2048, 4096] f32   -> flattened to [8192, 4096]
  blocks [16, 256, 256]  f32   (per-block [out, in])
  U      [4096, 64] f32, V [4096, 64] f32, bias [4096] f32, alpha [1] f32
  out    [4, 2048, 4096] f32

Sharding: data-parallel over tokens. Each of the 8 cores gets 1024 tokens
and the full (replicated) parameters; outputs are concatenated. No
collectives needed.

Per-core algorithm (T=1024 tokens, D=4096, R=64, NB=16, bi=bo=256):
  Setup (once): stage params; PE-transpose blocks -> blocksT bf16 (alpha
  folded in) and U -> uTb bf16 [65, D] whose row 64 is the bias (alpha on
  rows 0..63 only); V -> v_sb bf16.
  Steady state, halves of H=512 tokens:
    - 8 input DMAs [128, 2048] (sync queue)
    - PE transpose x -> xT bf16 [128, NK, H]; PSUM->SBUF copies alternate
      DVE/ACT
    - V-term: 32 matmuls N=H into tlr [64, H]; copied to tsb [65, H] whose
      row 64 is constant 1.0 (so the U matmul adds bias via its 65th
      contraction row)
    - per token-chunk: 8 PSUM acc tiles [128, 512]: 4 block-diagonal
      matmuls (K=128 each) + one U matmul (K=65, moving uTb N=512,
      stop=True); acc copied to osb (DVE/ACT alternating); one 2MB output
      DMA per token-chunk on the scalar queue.
  The in-NEFF repeat loop is a hardware For_i loop (hint_engines=PE), so
  NEFF size is independent of the repeat count used for timing.
"""

import numpy as np

import concourse.bacc as bacc
import concourse.mybir as mybir
import concourse.tile as tile
from concourse.bass_utils import run_bass_kernel_spmd
from concourse.masks import make_identity

F32 = mybir.dt.float32
F32R = mybir.dt.float32r
BF16 = mybir.dt.bfloat16

N_CORES = 8
D = 4096          # in = out features
R = 64            # low rank
NB = 16           # diagonal blocks
BI = 256          # block in/out size
NK = D // 128     # 32 i-chunks
T_CORE = 1024     # tokens per core
OC = 512          # output column chunk (one PSUM bank of f32)


def build(t_core: int = T_CORE, repeats: int = 1, io: str = "all",
          copy_mod: int = 2, copy_thresh: int = 1, out_q: str = "scalar",
          acc_bufs: int = 3, lr_bufs: int = 2, staggered: bool = False):
    do_in = io in ("all", "in")
    do_out = io in ("all", "out")
    nc = bacc.Bacc("TRN2", target_bir_lowering=False, debug=False)
    x = nc.declare_dram_parameter("x", [t_core, D], F32R, isOutput=False)
    blocks = nc.declare_dram_parameter("blocks", [NB, BI, BI], F32R, isOutput=False)
    U = nc.declare_dram_parameter("U", [D, R], F32R, isOutput=False)
    V = nc.declare_dram_parameter("V", [D, R], F32, isOutput=False)
    bias = nc.declare_dram_parameter("bias", [D], F32, isOutput=False)
    alpha = nc.declare_dram_parameter("alpha", [1], F32, isOutput=False)
    out = nc.declare_dram_parameter("out", [t_core, D], F32, isOutput=True)

    H = 512 if t_core % 512 == 0 else t_core   # tokens per half-pass
    n_h = t_core // H
    n_tc = H // 128               # 128-token chunks per half
    XC = 2048                     # input dma column chunk
    n_xc = D // XC

    with tile.TileContext(nc) as tc:
        with (
            tc.tile_pool(name="const", bufs=1) as cpool,
            tc.tile_pool(name="stage", bufs=1) as spool,
            tc.tile_pool(name="xnat", bufs=5) as xpool,
            tc.tile_pool(name="xT", bufs=2) as xTpool,
            tc.tile_pool(name="osb", bufs=2) as opool,
            tc.tile_pool(name="tsb", bufs=2) as tsbpool,
            tc.tile_pool(name="tp", bufs=3, space="PSUM") as tppool,
            tc.tile_pool(name="acc", bufs=acc_bufs, space="PSUM") as accpool,
            tc.tile_pool(name="lr", bufs=lr_bufs, space="PSUM") as lrpool,
        ):
            # ---------- constants ----------
            ident_f32 = spool.tile([128, 128], F32, tag="ident_f32")
            make_identity(nc, ident_f32[:])
            ident = cpool.tile([128, 128], F32R, tag="ident")
            nc.vector.tensor_copy(ident[:], ident_f32[:])

            ones_t = spool.tile([1, 128], F32, tag="ones")
            nc.vector.memset(ones_t[:], 1.0)
            alpha_row = spool.tile([1, 1], F32, tag="alpha_row")
            nc.sync.dma_start(alpha_row[:], alpha[None, :])
            # broadcast alpha to [128, 1] via rank-1 matmul
            alpha_col = cpool.tile([128, 1], F32, tag="alpha_col")
            a_ps = tppool.tile([128, 512], F32, tag="tp")
            nc.tensor.matmul(a_ps[:, :1], ones_t[:], alpha_row[:],
                             start=True, stop=True)
            nc.vector.tensor_copy(alpha_col[:], a_ps[:, :1])

            # ---------- params: blocksT, uTb(+bias), v_sb ----------
            blocksT = cpool.tile([128, NK, BI], BF16, tag="blocksT")
            uTb = cpool.tile([65, NK, 128], BF16, tag="uTb")
            v_sb = cpool.tile([128, NK, R], BF16, tag="v_sb")

            blk_view = blocks.rearrange("b (g p) i -> p (b g) i", p=128)

            def setup_blocks_round(rnd):
                blk_stage = spool.tile([128, NB, BI], F32R, tag="blk")
                nc.sync.dma_start(blk_stage[:],
                                  blk_view[:, rnd * NB:(rnd + 1) * NB, :])
                for bb_ in range(NB // 2):
                    b = rnd * (NB // 2) + bb_
                    for ihalf in range(2):
                        ki = 2 * b + ihalf
                        pt = tppool.tile([128, 512], F32R, tag="tp")
                        for g in range(2):
                            nc.tensor.transpose(
                                pt[:, g * 128:(g + 1) * 128],
                                blk_stage[:, 2 * bb_ + g,
                                          ihalf * 128:(ihalf + 1) * 128],
                                ident[:],
                            )
                        nc.vector.tensor_scalar_mul(
                            blocksT[:, ki, :], pt[:, :256], alpha_col[:, 0:1])

            setup_blocks_round(0)
            setup_blocks_round(1)

            v_stage = spool.tile([128, NK, R], F32, tag="uv")
            nc.sync.dma_start(v_stage[:], V.rearrange("(a p) r -> p a r", p=128))
            nc.vector.tensor_copy(v_sb[:], v_stage[:])

            u_stage = spool.tile([128, NK, R], F32R, tag="uv")
            nc.sync.dma_start(u_stage[:], U.rearrange("(a p) r -> p a r", p=128))
            for j in range(NK // 4):
                up = tppool.tile([128, 512], F32R, tag="tp")
                for q in range(4):
                    a = 4 * j + q
                    nc.tensor.transpose(
                        up[:R, q * 128:(q + 1) * 128], u_stage[:, a, :], ident[:])
                nc.vector.tensor_scalar_mul(
                    uTb[:R, 4 * j:4 * j + 4, :], up[:R, :], alpha_col[:R, 0:1])

            bias_row = spool.tile([1, NK, 128], F32, tag="blk")
            nc.sync.dma_start(bias_row[:], bias[None, :])
            nc.vector.tensor_copy(uTb[R:R + 1, :, :], bias_row[:])

            # ---------- steady state ----------
            rr = [0]

            def copy_rr(dst, src):
                if rr[0] % copy_mod < copy_thresh:
                    nc.vector.tensor_copy(dst, src)
                else:
                    nc.scalar.copy(dst, src)
                rr[0] += 1

            def one_pass():
                for h in range(n_h):
                    t0 = h * H
                    xts = []
                    for tcI in range(n_tc):
                        row = []
                        for q in range(n_xc):
                            xnat = xpool.tile([128, XC], F32R, tag="xnat")
                            if do_in or h == 0:
                                nc.sync.dma_start(
                                    xnat[:],
                                    x[t0 + tcI * 128: t0 + (tcI + 1) * 128,
                                      q * XC:(q + 1) * XC])
                            row.append(xnat)
                        xts.append(row)

                    xT = xTpool.tile([128, NK, H], BF16, tag="xT")
                    nkq = XC // 128           # ki chunks per xnat tile
                    for tcI in range(n_tc):
                        for g in range(NK // 4):
                            pt = tppool.tile([128, 512], F32R, tag="tp")
                            for q in range(4):
                                ki = 4 * g + q
                                src = xts[tcI][ki // nkq]
                                kk = ki % nkq
                                nc.tensor.transpose(
                                    pt[:, q * 128:(q + 1) * 128],
                                    src[:, kk * 128:(kk + 1) * 128],
                                    ident[:],
                                )
                            copy_rr(
                                xT[:, 4 * g:4 * g + 4,
                                   tcI * 128:(tcI + 1) * 128],
                                pt[:])

                    tlr = lrpool.tile([R, H], F32, tag="tlr")
                    for ki in range(NK):
                        nc.tensor.matmul(
                            tlr[:], v_sb[:, ki, :], xT[:, ki, :],
                            start=(ki == 0), stop=(ki == NK - 1),
                            skip_group_check=True,
                        )
                    tsb = tsbpool.tile([R + 1, H], BF16, tag="tsb")
                    nc.gpsimd.memset(tsb[R:R + 1, :], 1.0)
                    nc.vector.tensor_copy(tsb[:R, :], tlr[:])

                    for tcI in range(n_tc):
                        osb = opool.tile([128, D], F32, tag="osb")
                        for oc in range(D // OC):
                            acc = accpool.tile([128, OC], F32, tag="acc")
                            for b2 in range(2):
                                b = 2 * oc + b2
                                for ih in range(2):
                                    ki = 2 * b + ih
                                    nc.tensor.matmul(
                                        acc[:, b2 * 256:(b2 + 1) * 256],
                                        xT[:, ki, tcI * 128:(tcI + 1) * 128],
                                        blocksT[:, ki, :],
                                        start=(b2 == 0 and ih == 0), stop=False,
                                        skip_group_check=True,
                                    )
                            nc.tensor.matmul(
                                acc[:], tsb[:, tcI * 128:(tcI + 1) * 128],
                                uTb[:, 4 * oc:4 * oc + 4, :],
                                start=False, stop=True, skip_group_check=True,
                            )
                            copy_rr(osb[:, oc * OC:(oc + 1) * OC], acc[:])
                        if do_out:
                            oeng = nc.scalar if out_q == "scalar" else nc.sync
                            oeng.dma_start(
                                out[t0 + tcI * 128: t0 + (tcI + 1) * 128, :],
                                osb[:])

            if repeats == 1:
                one_pass()
            else:
                with tc.For_i(0, repeats, 1,
                              hint_engines=(mybir.EngineType.PE,),
                              staggered_reset=staggered):
                    one_pass()
    nc.compile()
    return nc


def check_waits(nc, verbose=True):
    bad = 0
    for fn in nc.m.functions:
        for bb in fn.blocks:
            for ins in bb.instructions:
                tname = type(ins).__name__
                if tname == "InstDrain":
                    continue
                nw = len(ins.sync_info.on_wait) if ins.sync_info else 0
                if tname == "InstEventSemaphore" and nw <= 2:
                    continue
                if nw > 1:
                    bad += 1
                    if verbose:
                        print("MULTI-WAIT", tname, ins.name,
                              [(w.ant_name, w.wait_value) for w in ins.sync_info.on_wait])
    return bad


_NC_CACHE = {}


def _get_nc(t_core, repeats=1):
    key = (t_core, repeats)
    if key not in _NC_CACHE:
        _NC_CACHE[key] = build(t_core, repeats)
    return _NC_CACHE[key]


def kernel(x, blocks, U, V, bias, alpha):
    batch_dims = x.shape[:-1]
    x_flat = np.ascontiguousarray(x.reshape(-1, D).astype(np.float32))
    n_tok = x_flat.shape[0]
    t_core = n_tok // N_CORES
    nc = _get_nc(t_core)

    blocks = np.ascontiguousarray(blocks, dtype=np.float32)
    U = np.ascontiguousarray(U, dtype=np.float32)
    V = np.ascontiguousarray(V, dtype=np.float32)
    bias = np.ascontiguousarray(bias, dtype=np.float32)
    alpha = np.ascontiguousarray(alpha, dtype=np.float32)

    in_maps = [
        {
            "x": x_flat[c * t_core:(c + 1) * t_core],
            "blocks": blocks, "U": U, "V": V, "bias": bias, "alpha": alpha,
        }
        for c in range(N_CORES)
    ]
    res = run_bass_kernel_spmd(nc, in_maps, list(range(N_CORES)))
    out = np.concatenate([res.results[c]["out"] for c in range(N_CORES)], axis=0)
    return out.reshape(*batch_dims, D)


# revision 21
# speedup vs baseline: 1.2257x; 1.2257x over previous
"""BlockDiagonalLowRankLinear Trainium2 kernel.

y = BlockDiag(blocks) @ x + U @ (V.T @ x), scaled by alpha, plus bias.

Shapes (full problem):
  x      [4, 2048, 4096] f32   -> flattened to [8192, 4096]
  blocks [16, 256, 256]  f32   (per-block [out, in])
  U      [4096, 64] f32, V [4096, 64] f32, bias [4096] f32, alpha [1] f32
  out    [4, 2048, 4096] f32

Sharding: data-parallel over tokens. Each of the 8 cores gets 1024 tokens
and the full (replicated) parameters; outputs are concatenated. No
collectives needed.

Per-core algorithm (T=1024 tokens, D=4096, R=64, NB=16, bi=bo=256):
  Setup (once): stage params; PE-transpose blocks -> blocksT bf16 (alpha
  folded in) and U -> uTb bf16 [65, D] whose row 64 is the bias (alpha on
  rows 0..63 only); V -> v_sb bf16.
  Steady state, halves of H=512 tokens:
    - 8 input DMAs [128, 2048] (sync queue)
    - PE transpose x -> xT bf16 [128, NK, H]; PSUM->SBUF copies alternate
      DVE/ACT
    - V-term: 32 matmuls N=H into tlr [64, H]; copied to tsb [65, H] whose
      row 64 is constant 1.0 (so the U matmul adds bias via its 65th
      contraction row)
    - per token-chunk: 8 PSUM acc tiles [128, 512]: 4 block-diagonal
      matmuls (K=128 each) + one U matmul (K=65, moving uTb N=512,
      stop=True); acc copied to osb (DVE/ACT alternating); one 2MB output
      DMA per token-chunk issued from the gpsimd SWDGE queue (keeps the
      Activation sequencer free for copies).
  The in-NEFF repeat loop is a hardware For_i loop (hint_engines=PE), so
  NEFF size is independent of the repeat count used for timing.
"""

import numpy as np

import concourse.bacc as bacc
import concourse.mybir as mybir
import concourse.tile as tile
from concourse.bass_utils import run_bass_kernel_spmd
from concourse.masks import make_identity

F32 = mybir.dt.float32
F32R = mybir.dt.float32r
BF16 = mybir.dt.bfloat16

N_CORES = 8
D = 4096          # in = out features
R = 64            # low rank
NB = 16           # diagonal blocks
BI = 256          # block in/out size
NK = D // 128     # 32 i-chunks
T_CORE = 1024     # tokens per core
OC = 512          # output column chunk (one PSUM bank of f32)


def build(t_core: int = T_CORE, repeats: int = 1, io: str = "all",
          copy_mod: int = 2, copy_thresh: int = 1, out_q: str = "gpsimd",
          acc_bufs: int = 3, lr_bufs: int = 2, staggered: bool = False,
          copy3: bool = False, bf16t: bool = False):
    do_in = io in ("all", "in")
    do_out = io in ("all", "out")
    nc = bacc.Bacc("TRN2", target_bir_lowering=False, debug=False)
    x = nc.declare_dram_parameter("x", [t_core, D], F32R, isOutput=False)
    blocks = nc.declare_dram_parameter("blocks", [NB, BI, BI], F32R, isOutput=False)
    U = nc.declare_dram_parameter("U", [D, R], F32R, isOutput=False)
    V = nc.declare_dram_parameter("V", [D, R], F32, isOutput=False)
    bias = nc.declare_dram_parameter("bias", [D], F32, isOutput=False)
    alpha = nc.declare_dram_parameter("alpha", [1], F32, isOutput=False)
    out = nc.declare_dram_parameter("out", [t_core, D], F32, isOutput=True)

    H = 512 if t_core % 512 == 0 else t_core   # tokens per half-pass
    n_h = t_core // H
    n_tc = H // 128               # 128-token chunks per half
    XC = 2048                     # input dma column chunk
    n_xc = D // XC

    with tile.TileContext(nc) as tc:
        with (
            tc.tile_pool(name="const", bufs=1) as cpool,
            tc.tile_pool(name="stage", bufs=1) as spool,
            tc.tile_pool(name="xnat", bufs=4 if bf16t else 5) as xpool,
            tc.tile_pool(name="xbf", bufs=4) as xbpool,
            tc.tile_pool(name="xT", bufs=2) as xTpool,
            tc.tile_pool(name="osb", bufs=2) as opool,
            tc.tile_pool(name="tsb", bufs=2) as tsbpool,
            tc.tile_pool(name="tp", bufs=3, space="PSUM") as tppool,
            tc.tile_pool(name="acc", bufs=acc_bufs, space="PSUM") as accpool,
            tc.tile_pool(name="lr", bufs=lr_bufs, space="PSUM") as lrpool,
        ):
            # ---------- constants ----------
            ident_f32 = spool.tile([128, 128], F32, tag="ident_f32")
            make_identity(nc, ident_f32[:])
            ident = cpool.tile([128, 128], F32R, tag="ident")
            nc.vector.tensor_copy(ident[:], ident_f32[:])

            ones_t = spool.tile([1, 128], F32, tag="ones")
            nc.vector.memset(ones_t[:], 1.0)
            alpha_row = spool.tile([1, 1], F32, tag="alpha_row")
            nc.sync.dma_start(alpha_row[:], alpha[None, :])
            # broadcast alpha to [128, 1] via rank-1 matmul
            alpha_col = cpool.tile([128, 1], F32, tag="alpha_col")
            a_ps = tppool.tile([128, 512], F32, tag="tp")
            nc.tensor.matmul(a_ps[:, :1], ones_t[:], alpha_row[:],
                             start=True, stop=True)
            nc.vector.tensor_copy(alpha_col[:], a_ps[:, :1])

            # ---------- params: blocksT, uTb(+bias), v_sb ----------
            blocksT = cpool.tile([128, NK, BI], BF16, tag="blocksT")
            uTb = cpool.tile([65, NK, 128], BF16, tag="uTb")
            v_sb = cpool.tile([128, NK, R], BF16, tag="v_sb")

            blk_view = blocks.rearrange("b (g p) i -> p (b g) i", p=128)

            def setup_blocks_round(rnd):
                blk_stage = spool.tile([128, NB, BI], F32R, tag="blk")
                nc.sync.dma_start(blk_stage[:],
                                  blk_view[:, rnd * NB:(rnd + 1) * NB, :])
                for bb_ in range(NB // 2):
                    b = rnd * (NB // 2) + bb_
                    for ihalf in range(2):
                        ki = 2 * b + ihalf
                        pt = tppool.tile([128, 512], F32R, tag="tp")
                        for g in range(2):
                            nc.tensor.transpose(
                                pt[:, g * 128:(g + 1) * 128],
                                blk_stage[:, 2 * bb_ + g,
                                          ihalf * 128:(ihalf + 1) * 128],
                                ident[:],
                            )
                        nc.vector.tensor_scalar_mul(
                            blocksT[:, ki, :], pt[:, :256], alpha_col[:, 0:1])

            setup_blocks_round(0)
            setup_blocks_round(1)

            v_stage = spool.tile([128, NK, R], F32, tag="uv")
            nc.sync.dma_start(v_stage[:], V.rearrange("(a p) r -> p a r", p=128))
            nc.vector.tensor_copy(v_sb[:], v_stage[:])

            u_stage = spool.tile([128, NK, R], F32R, tag="uv")
            nc.sync.dma_start(u_stage[:], U.rearrange("(a p) r -> p a r", p=128))
            for j in range(NK // 4):
                up = tppool.tile([128, 512], F32R, tag="tp")
                for q in range(4):
                    a = 4 * j + q
                    nc.tensor.transpose(
                        up[:R, q * 128:(q + 1) * 128], u_stage[:, a, :], ident[:])
                nc.vector.tensor_scalar_mul(
                    uTb[:R, 4 * j:4 * j + 4, :], up[:R, :], alpha_col[:R, 0:1])

            bias_row = spool.tile([1, NK, 128], F32, tag="blk")
            nc.sync.dma_start(bias_row[:], bias[None, :])
            nc.vector.tensor_copy(uTb[R:R + 1, :, :], bias_row[:])

            # ---------- steady state ----------
            rr = [0]

            def copy_rr(dst, src):
                if copy3:
                    k = rr[0] % 4
                    if k in (0, 2):
                        nc.vector.tensor_copy(dst, src)
                    elif k == 1:
                        nc.scalar.copy(dst, src)
                    else:
                        nc.gpsimd.tensor_copy(dst, src)
                elif rr[0] % copy_mod < copy_thresh:
                    nc.vector.tensor_copy(dst, src)
                else:
                    nc.scalar.copy(dst, src)
                rr[0] += 1

            def one_pass():
                for h in range(n_h):
                    t0 = h * H
                    xts = []
                    for tcI in range(n_tc):
                        row = []
                        for q in range(n_xc):
                            xnat = xpool.tile([128, XC], F32R, tag="xnat")
                            if do_in or h == 0:
                                nc.sync.dma_start(
                                    xnat[:],
                                    x[t0 + tcI * 128: t0 + (tcI + 1) * 128,
                                      q * XC:(q + 1) * XC])
                            row.append(xnat)
                        xts.append(row)

                    if bf16t:
                        xbs = []
                        for tcI in range(n_tc):
                            rowb = []
                            for q in range(n_xc):
                                xb = xbpool.tile([128, XC], BF16, tag="xbf")
                                if (rr[0] + tcI + q) % 2 == 0:
                                    nc.vector.tensor_copy(xb[:], xts[tcI][q][:])
                                else:
                                    nc.scalar.copy(xb[:], xts[tcI][q][:])
                                rowb.append(xb)
                            xbs.append(rowb)
                        xts = xbs

                    xT = xTpool.tile([128, NK, H], BF16, tag="xT")
                    nkq = XC // 128           # ki chunks per xnat tile
                    for tcI in range(n_tc):
                        for g in range(NK // 4):
                            pt = tppool.tile([128, 512],
                                             BF16 if bf16t else F32R, tag="tp")
                            for q in range(4):
                                ki = 4 * g + q
                                src = xts[tcI][ki // nkq]
                                kk = ki % nkq
                                nc.tensor.transpose(
                                    pt[:, q * 128:(q + 1) * 128],
                                    src[:, kk * 128:(kk + 1) * 128],
                                    ident[:],
                                )
                            copy_rr(
                                xT[:, 4 * g:4 * g + 4,
                                   tcI * 128:(tcI + 1) * 128],
                                pt[:])

                    tlr = lrpool.tile([R, H], F32, tag="tlr")
                    for ki in range(NK):
                        nc.tensor.matmul(
                            tlr[:], v_sb[:, ki, :], xT[:, ki, :],
                            start=(ki == 0), stop=(ki == NK - 1),
                            skip_group_check=True,
                        )
                    tsb = tsbpool.tile([R + 1, H], BF16, tag="tsb")
                    nc.gpsimd.memset(tsb[R:R + 1, :], 1.0)
                    nc.vector.tensor_copy(tsb[:R, :], tlr[:])

                    for tcI in range(n_tc):
                        osb = opool.tile([128, D], F32, tag="osb")
                        for oc in range(D // OC):
                            acc = accpool.tile([128, OC], F32, tag="acc")
                            for b2 in range(2):
                                b = 2 * oc + b2
                                for ih in range(2):
                                    ki = 2 * b + ih
                                    nc.tensor.matmul(
                                        acc[:, b2 * 256:(b2 + 1) * 256],
                                        xT[:, ki, tcI * 128:(tcI + 1) * 128],
                                        blocksT[:, ki, :],
                                        start=(b2 == 0 and ih == 0), stop=False,
                                        skip_group_check=True,
                                    )
                            nc.tensor.matmul(
                                acc[:], tsb[:, tcI * 128:(tcI + 1) * 128],
                                uTb[:, 4 * oc:4 * oc + 4, :],
                                start=False, stop=True, skip_group_check=True,
                            )
                            copy_rr(osb[:, oc * OC:(oc + 1) * OC], acc[:])
                        if do_out:
                            oeng = {"scalar": nc.scalar, "sync": nc.sync,
                                    "gpsimd": nc.gpsimd}[out_q]
                            oeng.dma_start(
                                out[t0 + tcI * 128: t0 + (tcI + 1) * 128, :],
                                osb[:])

            if repeats == 1:
                one_pass()
            else:
                with tc.For_i(0, repeats, 1,
                              hint_engines=(mybir.EngineType.PE,),
                              staggered_reset=staggered):
                    one_pass()
    nc.compile()
    return nc


def check_waits(nc, verbose=True):
    bad = 0
    for fn in nc.m.functions:
        for bb in fn.blocks:
            for ins in bb.instructions:
                tname = type(ins).__name__
                if tname == "InstDrain":
                    continue
                nw = len(ins.sync_info.on_wait) if ins.sync_info else 0
                if tname == "InstEventSemaphore" and nw <= 2:
                    continue
                if nw > 1:
                    bad += 1
                    if verbose:
                        print("MULTI-WAIT", tname, ins.name,
                              [(w.ant_name, w.wait_value) for w in ins.sync_info.on_wait])
    return bad


_NC_CACHE = {}


def _get_nc(t_core, repeats=1):
    key = (t_core, repeats)
    if key not in _NC_CACHE:
        _NC_CACHE[key] = build(t_core, repeats)
    return _NC_CACHE[key]


def kernel(x, blocks, U, V, bias, alpha):
    batch_dims = x.shape[:-1]
    x_flat = np.ascontiguousarray(x.reshape(-1, D).astype(np.float32))
    n_tok = x_flat.shape[0]
    t_core = n_tok // N_CORES
    nc = _get_nc(t_core)

    blocks = np.ascontiguousarray(blocks, dtype=np.float32)
    U = np.ascontiguousarray(U, dtype=np.float32)
    V = np.ascontiguousarray(V, dtype=np.float32)
    bias = np.ascontiguousarray(bias, dtype=np.float32)
    alpha = np.ascontiguousarray(alpha, dtype=np.float32)

    in_maps = [
        {
            "x": x_flat[c * t_core:(c + 1) * t_core],
            "blocks": blocks, "U": U, "V": V, "bias": bias, "alpha": alpha,
        }
        for c in range(N_CORES)
    ]
    res = run_bass_kernel_spmd(nc, in_maps, list(range(N_CORES)))
    out = np.concatenate([res.results[c]["out"] for c in range(N_CORES)], axis=0)
    return out.reshape(*batch_dims, D)
